# revision 1
# baseline (speedup 1.0000x reference)
"""NeuralMemory fast-weight recurrence on 8 Trainium2 NeuronCores.

Sharding: 8-way tensor-parallel over the memory dim M=2048 (m_s=256/core).
Per chunk: MLP forward, analytic MSE backward, gated fast-weight update,
re-forward. Cross-core: one bf16 AllReduce per chunk of the partial layer-2
activation (pred); the final `out` is returned as per-core partial sums that
the host adds (sum-sharded gather).

Numerics: bf16 matmul operands, fp32 PSUM accumulation. Weights are kept in
"Q-space" (divided by the running forget product c_j) so each update is a
single fused scaled-add (DVE scalar_tensor_tensor reading the gradient PSUM);
the forward applies c via ScalarEngine fused scale. Transposed layouts
(xT, hT, dpredT) are produced with PE-transposes (identity matmul) +
DVE/ACT PSUM evacuation; gW1's second layout comes from a swapped matmul.
"""
import numpy as np
import concourse.bacc as bacc
import concourse.mybir as mybir
import concourse.tile as tile
from concourse.bass_utils import run_bass_kernel_spmd

BF = mybir.dt.bfloat16
F32 = mybir.dt.float32
AF = mybir.ActivationFunctionType
ALU = mybir.AluOpType

NCORES = 8
B, L, D, M = 2, 2048, 2048, 2048
C = 128                 # reference CHUNK
NCH = L // C            # 16 chunks
T = B * C               # 256 tokens per chunk
MS = M // NCORES        # 256 per-core memory slice
KD = D // 128           # 16 tiles over D
KT = T // 128           # 2 tiles over tokens
KM = MS // 128          # 2 tiles over m_s
NN = D // 512           # 4 N-chunks of 512 over D
LR_MEMORY = 0.01


def build(no_ar=False):
    nc = bacc.Bacc("TRN2", target_bir_lowering=False, num_devices=NCORES)
    x = nc.dram_tensor("x", [B, L, D], F32, kind="ExternalInput")
    w0t_in = nc.dram_tensor("w0t", [D, MS], F32, kind="ExternalInput")
    w1t_in = nc.dram_tensor("w1t", [MS, D], F32, kind="ExternalInput")
    w1n_in = nc.dram_tensor("w1n", [D, MS], F32, kind="ExternalInput")
    lrfg_in = nc.dram_tensor("lrfg", [D, 2], F32, kind="ExternalInput")
    lrb_in = nc.dram_tensor("lrb", [1, 1], F32, kind="ExternalInput")
    fgb_in = nc.dram_tensor("fgb", [1, 1], F32, kind="ExternalInput")
    b0_in = nc.dram_tensor("b0", [1, MS], F32, kind="ExternalInput")
    b1d8_in = nc.dram_tensor("b1d8", [1, D], F32, kind="ExternalInput")
    ident_in = nc.dram_tensor("ident", [128, 128], F32, kind="ExternalInput")
    outp = nc.dram_tensor("outp", [B, L, D], F32, kind="ExternalOutput")

    with tile.TileContext(nc) as tc:
        with (
            tc.tile_pool(name="wp", bufs=1) as wp,          # persistent weights/consts
            tc.tile_pool(name="xp", bufs=2) as xp,          # per-chunk x streams (prefetched)
            tc.tile_pool(name="ap", bufs=2) as ap,          # loop-carried activations
            tc.tile_pool(name="tp", bufs=1) as tp,          # within-iteration temporaries
            tc.tile_pool(name="sp", bufs=2) as spool,       # tiny scalar tiles
            tc.tile_pool(name="psA", bufs=2, space="PSUM") as psA,   # [128,512]
            tc.tile_pool(name="psB", bufs=2, space="PSUM") as psB,   # [128,256]
            tc.tile_pool(name="psD", bufs=2, space="PSUM") as psD,   # small rows
            tc.tile_pool(name="psT", bufs=2, space="PSUM") as psT,   # transpose blocks
            tc.tile_pool(name="dr", bufs=2, space="DRAM") as dr,
        ):
            # ---------------- persistent weights (bf16, Q-space) ----------------
            q0t = wp.tile([128, KD * MS], BF, name="q0t")      # W0^T: d-tile i at cols i*MS
            q1t = wp.tile([128, KM * D], BF, name="q1t")       # W1^T: m-tile k at cols k*D
            q1n = wp.tile([128, KD * MS], BF, name="q1n")      # W1:   d-tile i at cols i*MS
            bk0 = wp.tile([128, MS], BF, name="bk0")           # row0 = qb0
            bk1 = wp.tile([128, D], BF, name="bk1")            # row0 = qb1/8
            lrfg = wp.tile([128, KD * 2], BF, name="lrfg")     # d-tile i at cols 2i..2i+1
            ones_row = wp.tile([128, 128], BF, name="ones_row")  # row0 = 1, rest 0
            ones_col = wp.tile([128, 1], BF, name="ones_col")    # all ones
            ident = wp.tile([128, 128], BF, name="ident")        # identity (PE transpose)
            lrb_sb = wp.tile([1, 1], F32, name="lrb_sb")
            fgb_sb = wp.tile([1, 1], F32, name="fgb_sb")

            w0t3 = w0t_in.rearrange("(i p) m -> i p m", p=128)
            w1t3 = w1t_in.rearrange("(k p) d -> k p d", p=128)
            w1n3 = w1n_in.rearrange("(i p) m -> i p m", p=128)
            lrfg3 = lrfg_in.rearrange("(i p) g -> i p g", p=128)
            for i in range(KD):
                nc.gpsimd.dma_start(q0t[:, i * MS:(i + 1) * MS], w0t3[i])
                nc.gpsimd.dma_start(q1n[:, i * MS:(i + 1) * MS], w1n3[i])
                nc.gpsimd.dma_start(lrfg[:, 2 * i:2 * i + 2], lrfg3[i])
            for k in range(KM):
                nc.gpsimd.dma_start(q1t[:, k * D:(k + 1) * D], w1t3[k])
            nc.gpsimd.memset(bk0[:], 0.0)
            nc.gpsimd.memset(bk1[:], 0.0)
            nc.gpsimd.dma_start(bk0[0:1, :], b0_in[:])
            nc.gpsimd.dma_start(bk1[0:1, :], b1d8_in[:])
            nc.gpsimd.memset(ones_row[:], 0.0)
            nc.vector.memset(ones_row[0:1, :], 1.0)
            nc.vector.memset(ones_col[:], 1.0)
            nc.gpsimd.dma_start(ident[:], ident_in[:])
            nc.sync.dma_start(lrb_sb[:], lrb_in[:])
            nc.sync.dma_start(fgb_sb[:], fgb_in[:])

            # running forget product c (scalar state), starts at 1
            c11 = spool.tile([1, 1], F32, name="c11")
            c_bc = spool.tile([128, 1], F32, name="c_bc")
            nc.vector.memset(c11[:], 1.0)
            nc.gpsimd.partition_broadcast(c_bc[:], c11[:])

            # ---------------- helpers ----------------
            def pe_transpose(dst, dst_col, src, src_col, j, who, ei):
                """dst[:, dst_col:+128] = src[:, src_col:+128].T via PE."""
                pt = psT.tile([128, 128], BF, name=f"tp{who}_{j}_{dst_col}", tag="psT")
                nc.tensor.transpose(pt[:], src[:, src_col:src_col + 128], ident[:])
                if ei == 0:
                    nc.vector.tensor_copy(dst[:, dst_col:dst_col + 128], pt[:])
                else:
                    nc.scalar.copy(dst[:, dst_col:dst_col + 128], pt[:])

            def load_chunk(j):
                xb = xp.tile([128, KT * D], BF, name=f"xb{j}", tag="xb")
                for t in range(KT):
                    nc.gpsimd.dma_start(xb[:, t * D:(t + 1) * D], x[t, j * C:(j + 1) * C, :])
                return xb

            def transpose_x(xb, j):
                xT = xp.tile([128, KD * T], BF, name=f"xT{j}", tag="xT")
                for t in range(KT):
                    for i in range(KD):
                        pe_transpose(xT, i * T + t * 128, xb, t * D + i * 128, j, "x",
                                     (t * KD + i) % 2)
                return xT

            def mm1(xT, cb, want_dsilu, j, pfx, pool):
                h = pool.tile([128, KT * MS], BF, name=f"h{pfx}_{j}", tag=f"h{pfx}")
                hp = None
                if want_dsilu:
                    hp = pool.tile([128, KT * MS], BF, name=f"hp_{j}", tag="hp")
                pts = []
                for t in range(KT):
                    pt = psB.tile([128, MS], F32, name=f"psh{pfx}_{j}_{t}", tag="psB")
                    for i in range(KD):
                        nc.tensor.matmul(pt[:], xT[:, i * T + t * 128:i * T + (t + 1) * 128],
                                         q0t[:, i * MS:(i + 1) * MS],
                                         start=(i == 0), stop=False)
                    nc.tensor.matmul(pt[:], ones_row[:], bk0[:], start=False, stop=True)
                    pts.append(pt)
                for t in range(KT):  # group by ACT function to limit table reloads
                    nc.scalar.activation(h[:, t * MS:(t + 1) * MS], pts[t][:], AF.Silu,
                                         scale=cb[:, 0:1])
                if want_dsilu:
                    for t in range(KT):
                        nc.scalar.activation(hp[:, t * MS:(t + 1) * MS], pts[t][:],
                                             AF.Derivative_silu, scale=cb[:, 0:1])
                return h, hp

            def transpose_h(h, j, pfx, pool):
                hT = pool.tile([128, KM * T], BF, name=f"hT{pfx}_{j}", tag=f"hT{pfx}")
                for t in range(KT):
                    for k in range(KM):
                        pe_transpose(hT, k * T + t * 128, h, t * MS + k * 128, j,
                                     f"h{pfx}", (t + k) % 2)
                return hT

            def mm2(hT, cb, out_dtype, j, pfx, pool):
                o = pool.tile([128, KT * D], out_dtype, name=f"o{pfx}_{j}", tag=f"o{pfx}")
                for t in range(KT):
                    for n in range(NN):
                        pt = psA.tile([128, 512], F32, name=f"psp{pfx}_{j}_{t}_{n}", tag="psA")
                        for k in range(KM):
                            nc.tensor.matmul(pt[:], hT[:, k * T + t * 128:k * T + (t + 1) * 128],
                                             q1t[:, k * D + n * 512:k * D + (n + 1) * 512],
                                             start=(k == 0), stop=False)
                        nc.tensor.matmul(pt[:], ones_row[:], bk1[:, n * 512:(n + 1) * 512],
                                         start=False, stop=True)
                        nc.scalar.activation(o[:, t * D + n * 512:t * D + (n + 1) * 512], pt[:],
                                             AF.Copy, scale=cb[:, 0:1])
                return o

            def issue_ar(predp, j):
                arin = dr.tile([T, D], BF, name=f"arin{j}", tag="arin")
                for t in range(KT):
                    nc.gpsimd.dma_start(arin[t * 128:(t + 1) * 128, :],
                                        predp[:, t * D:(t + 1) * D])
                if no_ar:
                    return arin
                arout = dr.tile([T, D], BF, name=f"arout{j}", tag="arout",
                                addr_space="Shared")
                nc.gpsimd.collective_compute(
                    "AllReduce", ALU.add, replica_groups=[list(range(NCORES))],
                    ins=[arin.opt()], outs=[arout.opt()])
                return arout

            # ---------------- prologue: chunk 0 forward under P_0 ----------------
            xb_c = load_chunk(0)
            xT_c = transpose_x(xb_c, 0)
            h1_c, hp1_c = mm1(xT_c, c_bc, True, 0, "1", ap)
            h1T_c = transpose_h(h1_c, 0, "1", ap)
            predp = mm2(h1T_c, c_bc, BF, 0, "p", ap)
            ar_cur = issue_ar(predp, 0)

            # ---------------- main loop ----------------
            for j in range(NCH):
                last = (j == NCH - 1)

                # gates from chunk j (independent of the AllReduce)
                g2a = psD.tile([1, T], F32, name=f"g2a_{j}", tag="psD")
                g2b = psD.tile([1, T], F32, name=f"g2b_{j}", tag="psD")
                for i in range(KD):
                    nc.tensor.matmul(g2a[:], lrfg[:, 2 * i:2 * i + 1], xT_c[:, i * T:(i + 1) * T],
                                     start=(i == 0), stop=(i == KD - 1))
                for i in range(KD):
                    nc.tensor.matmul(g2b[:], lrfg[:, 2 * i + 1:2 * i + 2], xT_c[:, i * T:(i + 1) * T],
                                     start=(i == 0), stop=(i == KD - 1))
                sigl = spool.tile([1, T], F32, name=f"sigl{j}", tag="sigl")
                lsum = spool.tile([1, 1], F32, name=f"lsum{j}", tag="lsum")
                nc.scalar.activation(sigl[:], g2a[:], AF.Sigmoid, bias=lrb_sb[0:1, 0:1],
                                     accum_out=lsum[:])
                fparts = spool.tile([1, 2], F32, name=f"fparts{j}", tag="fparts")
                for b in range(B):
                    r = spool.tile([1, 1], F32, name=f"zfr{j}_{b}", tag=f"zfr{b}")
                    nc.vector.tensor_reduce(r[:], g2b[0:1, b * C:(b + 1) * C],
                                            mybir.AxisListType.X, ALU.add)
                    nc.scalar.activation(fparts[:, b:b + 1], r[:], AF.Sigmoid,
                                         bias=fgb_sb[0:1, 0:1], scale=1.0 / C)
                f11 = spool.tile([1, 1], F32, name=f"f11_{j}", tag="f11")
                nc.vector.tensor_reduce(f11[:], fparts[:], mybir.AxisListType.X, ALU.add)
                nc.vector.tensor_scalar_mul(f11[:], f11[:], 0.5)

                # scalars: c'=c*f ; s1 = LR*2/(N*T)*lsum/c' ; s0 = s1*c ; negated
                cn11 = spool.tile([1, 1], F32, name=f"cn{j}", tag="cn11")
                nc.vector.tensor_tensor(cn11[:], c11[:], f11[:], ALU.mult)
                rcn = spool.tile([1, 1], F32, name=f"rcn{j}", tag="rcn")
                nc.vector.reciprocal(rcn[:], cn11[:])
                negs = spool.tile([1, 1], F32, name=f"negs{j}", tag="negs")
                nc.vector.tensor_tensor(negs[:], lsum[:], rcn[:], ALU.mult)
                nc.vector.tensor_scalar_mul(negs[:], negs[:],
                                            float(-LR_MEMORY * 2.0 / (T * D) / T))
                negs8 = spool.tile([1, 1], F32, name=f"negs8_{j}", tag="negs8")
                nc.vector.tensor_scalar_mul(negs8[:], negs[:], 1.0 / 8.0)
                negs0 = spool.tile([1, 1], F32, name=f"negs0_{j}", tag="negs0")
                nc.vector.tensor_tensor(negs0[:], negs[:], c11[:], ALU.mult)
                negs_bc = spool.tile([128, 1], F32, name=f"negsbc{j}", tag="negs_bc")
                nc.gpsimd.partition_broadcast(negs_bc[:], negs[:])
                negs0_bc = spool.tile([128, 1], F32, name=f"negs0bc{j}", tag="negs0_bc")
                nc.gpsimd.partition_broadcast(negs0_bc[:], negs0[:])
                cn_bc = spool.tile([128, 1], F32, name=f"cnbc{j}", tag="cn_bc")
                nc.gpsimd.partition_broadcast(cn_bc[:], cn11[:])

                # prefetch next chunk (DMA + PE transposes)
                if not last:
                    xb_n = load_chunk(j + 1)
                    xT_n = transpose_x(xb_n, j + 1)

                # AllReduce result -> SBUF ; dpred = pred_full - x (raw)
                pred_full = tp.tile([128, KT * D], BF, name=f"pf{j}", tag="pf")
                for t in range(KT):
                    nc.sync.dma_start(pred_full[:, t * D:(t + 1) * D],
                                      ar_cur[t * 128:(t + 1) * 128, :])
                dpred = tp.tile([128, KT * D], BF, name=f"dp{j}", tag="dp")
                for t in range(KT):
                    nc.vector.tensor_sub(dpred[:, t * D:(t + 1) * D],
                                         pred_full[:, t * D:(t + 1) * D],
                                         xb_c[:, t * D:(t + 1) * D])
                dpredT = tp.tile([128, KD * T], BF, name=f"dpT{j}", tag="dpT")
                for t in range(KT):
                    for i in range(KD):
                        pe_transpose(dpredT, i * T + t * 128, dpred, t * D + i * 128, j,
                                     "dp", (t * KD + i) % 2)

                # dh = dpredT(lhsT) x q1n_OLD -> [T x m_s] ; dh_pre = dh * hp'
                dhp = tp.tile([128, KT * MS], BF, name=f"dhp{j}", tag="dhp")
                for t in range(KT):
                    pt = psB.tile([128, MS], F32, name=f"psdh{j}_{t}", tag="psB")
                    for i in range(KD):
                        nc.tensor.matmul(pt[:], dpredT[:, i * T + t * 128:i * T + (t + 1) * 128],
                                         q1n[:, i * MS:(i + 1) * MS],
                                         start=(i == 0), stop=(i == KD - 1))
                    nc.vector.tensor_tensor(dhp[:, t * MS:(t + 1) * MS], pt[:],
                                            hp1_c[:, t * MS:(t + 1) * MS], ALU.mult)

                # gW1t = h1(lhsT) x dpred ; fused update of q1t (x -s1)   [after dh]
                for k in range(KM):
                    for n in range(NN):
                        pt = psA.tile([128, 512], F32, name=f"psg1_{j}_{k}_{n}", tag="psA")
                        for t in range(KT):
                            nc.tensor.matmul(pt[:],
                                             h1_c[:, t * MS + k * 128:t * MS + (k + 1) * 128],
                                             dpred[:, t * D + n * 512:t * D + (n + 1) * 512],
                                             start=(t == 0), stop=(t == KT - 1))
                        sl = slice(k * D + n * 512, k * D + (n + 1) * 512)
                        nc.vector.scalar_tensor_tensor(q1t[:, sl], pt[:], negs_bc[:, 0:1],
                                                       q1t[:, sl], ALU.mult, ALU.add)
                # gW1n = dpred(lhsT) x h1 ; fused update of q1n (x -s1)
                for i in range(KD):
                    pt = psB.tile([128, MS], F32, name=f"psg1n_{j}_{i}", tag="psB")
                    for t in range(KT):
                        nc.tensor.matmul(pt[:], dpred[:, t * D + i * 128:t * D + (i + 1) * 128],
                                         h1_c[:, t * MS:(t + 1) * MS],
                                         start=(t == 0), stop=(t == KT - 1))
                    sl = slice(i * MS, (i + 1) * MS)
                    nc.vector.scalar_tensor_tensor(q1n[:, sl], pt[:], negs_bc[:, 0:1],
                                                   q1n[:, sl], ALU.mult, ALU.add)
                # gb1 -> bk1 row0 (scale -s1/8), chunked [1,512]
                for n in range(NN):
                    gb1p = psD.tile([1, 512], F32, name=f"gb1_{j}_{n}", tag="psD")
                    for t in range(KT):
                        nc.tensor.matmul(gb1p[:], ones_col[:],
                                         dpred[:, t * D + n * 512:t * D + (n + 1) * 512],
                                         start=(t == 0), stop=(t == KT - 1))
                    nc.vector.scalar_tensor_tensor(bk1[0:1, n * 512:(n + 1) * 512], gb1p[:],
                                                   negs8[0:1, 0:1],
                                                   bk1[0:1, n * 512:(n + 1) * 512],
                                                   ALU.mult, ALU.add)

                # gW0t = x(lhsT) x dh_pre -> [D x m_s] ; fused update q0t (x -s0)
                for i in range(KD):
                    pt = psB.tile([128, MS], F32, name=f"psg0_{j}_{i}", tag="psB")
                    for t in range(KT):
                        nc.tensor.matmul(pt[:], xb_c[:, t * D + i * 128:t * D + (i + 1) * 128],
                                         dhp[:, t * MS:(t + 1) * MS],
                                         start=(t == 0), stop=(t == KT - 1))
                    sl = slice(i * MS, (i + 1) * MS)
                    nc.vector.scalar_tensor_tensor(q0t[:, sl], pt[:], negs0_bc[:, 0:1],
                                                   q0t[:, sl], ALU.mult, ALU.add)
                # gb0 -> bk0 row0 (scale -s0)
                gb0p = psD.tile([1, MS], F32, name=f"gb0_{j}", tag="psD")
                for t in range(KT):
                    nc.tensor.matmul(gb0p[:], ones_col[:], dhp[:, t * MS:(t + 1) * MS],
                                     start=(t == 0), stop=(t == KT - 1))
                nc.vector.scalar_tensor_tensor(bk0[0:1, :], gb0p[:], negs0[0:1, 0:1],
                                               bk0[0:1, :], ALU.mult, ALU.add)

                # ---- forward under P_{j+1}: out_j (f32 partial) and pred_{j+1} ----
                h2, _ = mm1(xT_c, cn_bc, False, j, "2", tp)
                h2T = transpose_h(h2, j, "2", tp)
                outsb = mm2(h2T, cn_bc, F32, j, "o", tp)
                for t in range(KT):
                    nc.sync.dma_start(outp[t, j * C:(j + 1) * C, :],
                                      outsb[:, t * D:(t + 1) * D])
                if not last:
                    h1_n, hp1_n = mm1(xT_n, cn_bc, True, j + 1, "1", ap)
                    h1T_n = transpose_h(h1_n, j + 1, "1", ap)
                    predp = mm2(h1T_n, cn_bc, BF, j + 1, "p", ap)
                    ar_cur = issue_ar(predp, j + 1)
                    xb_c, xT_c = xb_n, xT_n
                    h1_c, hp1_c, h1T_c = h1_n, hp1_n, h1T_n
                c11, c_bc = cn11, cn_bc
    nc.compile()
    return nc


_NC_CACHE = None


def _get_nc():
    global _NC_CACHE
    if _NC_CACHE is None:
        _NC_CACHE = build()
    return _NC_CACHE


def make_in_maps(x, W0, b0, W1, b1, lr_w, lr_b, fg_w, fg_b):
    x = np.ascontiguousarray(np.asarray(x, np.float32))
    W0 = np.asarray(W0, np.float32)
    W1 = np.asarray(W1, np.float32)
    lrfg = np.ascontiguousarray(
        np.stack([np.asarray(lr_w, np.float32)[0], np.asarray(fg_w, np.float32)[0]], axis=1))
    ident = np.eye(128, dtype=np.float32)
    in_maps = []
    for s in range(NCORES):
        sl = slice(s * MS, (s + 1) * MS)
        in_maps.append({
            "x": x,
            "w0t": np.ascontiguousarray(W0[sl, :].T),
            "w1t": np.ascontiguousarray(W1[:, sl].T),
            "w1n": np.ascontiguousarray(W1[:, sl]),
            "lrfg": lrfg,
            "lrb": np.asarray(lr_b, np.float32).reshape(1, 1),
            "fgb": np.asarray(fg_b, np.float32).reshape(1, 1),
            "b0": np.ascontiguousarray(np.asarray(b0, np.float32)[sl].reshape(1, MS)),
            "b1d8": np.ascontiguousarray((np.asarray(b1, np.float32) / 8.0).reshape(1, D)),
            "ident": ident,
        })
    return in_maps


def run(inputs, **kw):
    nc = _get_nc()
    in_maps = make_in_maps(**inputs)
    res = run_bass_kernel_spmd(nc, in_maps, core_ids=list(range(NCORES)), **kw)
    out = np.zeros((B, L, D), np.float32)
    for r in res.results:
        out += r["outp"]
    return out, res


def kernel(**inputs) -> np.ndarray:
    out, _ = run(inputs)
    return out



# revision 16
# speedup vs baseline: 1.9748x; 1.9748x over previous
"""NeuralMemory fast-weight recurrence on 8 Trainium2 NeuronCores.

Sharding: 8-way tensor-parallel over memory dim M=2048 (m_s=256/core).
One bf16 AllReduce per chunk carries the partial pred with x/8
pre-subtracted, so the AR output IS dpred = pred - x; the d-partitioned
copy (dpredT) is read back via hardware XBAR DMA transpose.
Per chunk the serial chain is:
  AR_j -> dh' -> gW0/q0t update -> mm1_{j+1} -> silu -> hT ->
  gW1t/q1t + gb1 updates -> mm2 pred_{j+1} -> AR_{j+1}
while gW1n/q1n update, out_{j-1} re-forward, gates, and x prefetch run
inside the AllReduce wait window.

- Host passes x pre-converted to bf16 in BOTH layouts (xb16 [B,L,D],
  xt16 [D,B,L]) - no casting DMAs, no PE transposes for x.
- Weights kept in Q-space (divided by running forget product c) so each
  update is one fused scaled-add reading the gradient PSUM.
- Gradient/update PSUMs use [128,1024] waves to halve DVE evac count.
- Output partials stored bf16; host sums 8 cores in f32.
"""
import numpy as np
import concourse.bacc as bacc
import concourse.mybir as mybir
import concourse.tile as tile
from concourse.bass_utils import run_bass_kernel_spmd

BF = mybir.dt.bfloat16
F32 = mybir.dt.float32
AF = mybir.ActivationFunctionType
ALU = mybir.AluOpType

NCORES = 8
B, L, D, M = 2, 2048, 2048, 2048
C = 128                 # reference CHUNK
NCH = L // C            # 16 chunks
T = B * C               # 256 tokens per chunk
MS = M // NCORES        # 256 per-core memory slice
KD = D // 128           # 16 tiles over D
KT = T // 128           # 2 tiles over tokens
KM = MS // 128          # 2 tiles over m_s
LR_MEMORY = 0.01


def build(no_ar=False):
    nc = bacc.Bacc("TRN2", target_bir_lowering=False, num_devices=NCORES)
    xb16_in = nc.dram_tensor("xb16", [B, L, D], BF, kind="ExternalInput")
    xt16_in = nc.dram_tensor("xt16", [D, B, L], BF, kind="ExternalInput")
    w0t_in = nc.dram_tensor("w0t", [D, MS], BF, kind="ExternalInput")
    w1t_in = nc.dram_tensor("w1t", [MS, D], BF, kind="ExternalInput")
    w1n_in = nc.dram_tensor("w1n", [D, MS], BF, kind="ExternalInput")
    lrfg_in = nc.dram_tensor("lrfg", [D, 2], BF, kind="ExternalInput")
    lrb_in = nc.dram_tensor("lrb", [1, 1], F32, kind="ExternalInput")
    fgb_in = nc.dram_tensor("fgb", [1, 1], F32, kind="ExternalInput")
    b0_in = nc.dram_tensor("b0", [1, MS], BF, kind="ExternalInput")
    b1d8_in = nc.dram_tensor("b1d8", [1, D], BF, kind="ExternalInput")
    ident_in = nc.dram_tensor("ident", [128, 128], BF, kind="ExternalInput")
    outp = nc.dram_tensor("outp", [B, L, D], BF, kind="ExternalOutput")

    xb4 = xb16_in.rearrange("b (nj p) d -> nj p b d", p=C)        # [NCH,128,B,D]
    xt4 = xt16_in.rearrange("(i p) b (nj c) -> nj p i b c", p=128, c=C)
    op4 = outp.rearrange("b (nj p) d -> nj p b d", p=C)

    with tile.TileContext(nc) as tc:
        with (
            tc.tile_pool(name="wp", bufs=1) as wp,            # persistent weights
            tc.tile_pool(name="xp", bufs=4) as xp,            # x chunk streams
            tc.tile_pool(name="x8p", bufs=2) as x8p,          # x/8 staging
            tc.tile_pool(name="hp_", bufs=2) as hpool,        # h/hp/dhp
            tc.tile_pool(name="htp", bufs=2) as htp,          # hT
            tc.tile_pool(name="dpp", bufs=2) as dpp,          # dpred/dpredT sbuf
            tc.tile_pool(name="arp", bufs=1) as arp,          # arin staging sbuf
            tc.tile_pool(name="otp", bufs=1) as otp,          # out/h2/hT2 staging
            tc.tile_pool(name="sp", bufs=2) as spool,         # tiny scalar tiles
            tc.tile_pool(name="psB", bufs=2, space="PSUM") as psB,  # [128,1024]
            tc.tile_pool(name="psA", bufs=2, space="PSUM") as psA,  # [128,512]
            tc.tile_pool(name="psZ", bufs=1, space="PSUM") as psZ,  # z0 accum
            tc.tile_pool(name="psT", bufs=1, space="PSUM") as psT,  # transposes
            tc.tile_pool(name="dr", bufs=2, space="DRAM") as dr,
        ):
            # ---------------- persistent weights (bf16, Q-space) ----------------
            q0t = wp.tile([128, KD * MS], BF, name="q0t")     # W0^T: d-tile i at i*MS
            q1t = wp.tile([128, KM * D], BF, name="q1t")      # W1^T: m-tile k at k*D
            q1n = wp.tile([128, KD * MS], BF, name="q1n")     # W1:   d-tile i at i*MS
            bk0 = wp.tile([128, MS], BF, name="bk0")          # row0 = qb0
            bk1 = wp.tile([128, D], BF, name="bk1")           # row0 = qb1/8
            lrfg = wp.tile([128, KD * 2], BF, name="lrfg")
            ones_row = wp.tile([128, 128], BF, name="ones_row")  # row0 = 1, rest 0
            ones_col = wp.tile([128, 1], BF, name="ones_col")    # all ones
            ident = wp.tile([128, 128], BF, name="ident")
            lrb_sb = wp.tile([1, 1], F32, name="lrb_sb")
            fgb_sb = wp.tile([1, 1], F32, name="fgb_sb")

            nc.sync.dma_start(q0t.rearrange("p (i m) -> p i m", m=MS),
                              w0t_in.rearrange("(i p) m -> p i m", p=128))
            nc.sync.dma_start(q1n.rearrange("p (i m) -> p i m", m=MS),
                              w1n_in.rearrange("(i p) m -> p i m", p=128))
            nc.sync.dma_start(q1t.rearrange("p (k d) -> p k d", d=D),
                              w1t_in.rearrange("(k p) d -> p k d", p=128))
            nc.sync.dma_start(lrfg.rearrange("p (i g) -> p i g", g=2),
                              lrfg_in.rearrange("(i p) g -> p i g", p=128))
            nc.gpsimd.memset(bk0[:], 0.0)
            nc.gpsimd.memset(bk1[:], 0.0)
            nc.sync.dma_start(bk0[0:1, :], b0_in[:])
            nc.sync.dma_start(bk1[0:1, :], b1d8_in[:])
            nc.gpsimd.memset(ones_row[:], 0.0)
            nc.vector.memset(ones_row[0:1, :], 1.0)
            nc.vector.memset(ones_col[:], 1.0)
            nc.sync.dma_start(ident[:], ident_in[:])
            nc.sync.dma_start(lrb_sb[:], lrb_in[:])
            nc.sync.dma_start(fgb_sb[:], fgb_in[:])

            # running forget product c (scalar state), starts at 1
            c11 = spool.tile([1, 1], F32, name="c11")
            c_bc = spool.tile([128, 1], F32, name="c_bc")
            nc.vector.memset(c11[:], 1.0)
            nc.gpsimd.partition_broadcast(c_bc[:], c11[:])

            # ---------------- helpers ----------------
            def load_x(j):
                xb = xp.tile([128, KT * D], BF, name=f"xb{j}", tag="xb")
                xT = xp.tile([128, KD * T], BF, name=f"xT{j}", tag="xT")
                nc.sync.dma_start(xb.rearrange("p (t d) -> p t d", d=D), xb4[j])
                xTv = xT.rearrange("p (i b c) -> p i b c", b=B, c=C)
                for b in range(B):
                    nc.sync.dma_start(xTv[:, :, b, :], xt4[j][:, :, b, :])
                return xb, xT

            def make_x8(xb, j):
                xb8 = x8p.tile([128, KT * D], BF, name=f"xb8_{j}", tag="xb8")
                nc.gpsimd.tensor_scalar_mul(xb8[:], xb[:], 0.125)
                return xb8

            def mm1_psum(xT, tag, j, pool, ptag):
                """z0 Q-space pre-activation [t(2 tiles), MS] in one psum tile."""
                ps = pool.tile([128, KT * MS], F32, name=f"z0{tag}{j}", tag=ptag)
                for t in range(KT):
                    sl = slice(t * MS, (t + 1) * MS)
                    for i in range(KD):
                        nc.tensor.matmul(ps[:, sl],
                                         xT[:, i * T + t * 128:i * T + (t + 1) * 128],
                                         q0t[:, i * MS:(i + 1) * MS],
                                         start=(i == 0), stop=False)
                    nc.tensor.matmul(ps[:, sl], ones_row[:], bk0[:],
                                     start=False, stop=True)
                return ps

            def transpose_h(h, j, tag):
                hT = htp.tile([128, KM * T], BF, name=f"hT{tag}{j}", tag=f"hT{tag}")
                for t in range(KT):
                    for k in range(KM):
                        pt = psT.tile([128, 128], BF, name=f"ptr{tag}{j}{t}{k}",
                                      tag="psT")
                        nc.tensor.transpose(
                            pt[:], h[:, t * MS + k * 128:t * MS + (k + 1) * 128],
                            ident[:])
                        if (t + k) % 2 == 0:
                            nc.vector.tensor_copy(
                                hT[:, k * T + t * 128:k * T + (t + 1) * 128], pt[:])
                        else:
                            nc.scalar.copy(hT[:, k * T + t * 128:k * T + (t + 1) * 128],
                                           pt[:])
                return hT

            def mm2_pred(hT, cnb, xb8_n, j):
                """pred partial [T, D] with Q-scale cn and -x/8 folded into the
                evac; [128,1024] waves -> 4 DVE evacs."""
                asb = arp.tile([128, KT * D], BF, name=f"arA{j}", tag="arA")
                for t in range(KT):
                    for w in range(2):
                        ps = psB.tile([128, 1024], F32, name=f"pp{j}_{t}_{w}",
                                      tag="psB")
                        for s in range(2):
                            n = 2 * w + s
                            sl = slice(s * 512, (s + 1) * 512)
                            for k in range(KM):
                                nc.tensor.matmul(
                                    ps[:, sl],
                                    hT[:, k * T + t * 128:k * T + (t + 1) * 128],
                                    q1t[:, k * D + n * 512:k * D + (n + 1) * 512],
                                    start=(k == 0), stop=False)
                            nc.tensor.matmul(ps[:, sl], ones_row[:],
                                             bk1[:, n * 512:(n + 1) * 512],
                                             start=False, stop=True)
                        sl2 = slice(t * D + w * 1024, t * D + (w + 1) * 1024)
                        nc.vector.scalar_tensor_tensor(asb[:, sl2], ps[:],
                                                       cnb[:, 0:1], xb8_n[:, sl2],
                                                       ALU.mult, ALU.subtract)
                return asb

            def issue_ars(asb, j):
                arA = dr.tile([T, D], BF, name=f"drA{j}", tag="drA")
                nc.sync.dma_start(arA.rearrange("(t p) d -> p t d", p=128),
                                  asb.rearrange("p (t d) -> p t d", d=D))
                if no_ar:
                    return arA
                aoA = dr.tile([T, D], BF, name=f"aoA{j}", tag="doA",
                              addr_space="Shared")
                nc.gpsimd.collective_compute(
                    "AllReduce", ALU.add, replica_groups=[list(range(NCORES))],
                    ins=[arA.opt()], outs=[aoA.opt()])
                return aoA

            def read_ar(aoA, j):
                dpT = dpp.tile([128, KD * T], BF, name=f"dpT{j}", tag="dpT")
                dp = dpp.tile([128, KT * D], BF, name=f"dp{j}", tag="dp")
                # dpredT tiles via hardware XBAR transpose straight from the
                # AllReduce output (dh' needs d-partitioned dpred first)
                for i in range(KD):
                    eng = nc.scalar if i % 2 == 0 else nc.sync
                    eng.dma_start(dpT[:, i * T:(i + 1) * T],
                                  aoA[:, i * 128:(i + 1) * 128], transpose=True)
                nc.sync.dma_start(dp.rearrange("p (t d) -> p t d", d=D),
                                  aoA.rearrange("(t p) d -> p t d", p=128))
                return dp, dpT

            def gates(xT, j):
                g2a = psA.tile([1, T], F32, name=f"g2a_{j}", tag="psA")
                g2b = psA.tile([1, T], F32, name=f"g2b_{j}", tag="psA")
                for i in range(KD):
                    nc.tensor.matmul(g2a[:], lrfg[:, 2 * i:2 * i + 1],
                                     xT[:, i * T:(i + 1) * T],
                                     start=(i == 0), stop=(i == KD - 1))
                for i in range(KD):
                    nc.tensor.matmul(g2b[:], lrfg[:, 2 * i + 1:2 * i + 2],
                                     xT[:, i * T:(i + 1) * T],
                                     start=(i == 0), stop=(i == KD - 1))
                sigl = spool.tile([1, T], F32, name=f"sigl{j}", tag="sigl")
                lsum = spool.tile([1, 1], F32, name=f"lsum{j}", tag="lsum")
                nc.scalar.activation(sigl[:], g2a[:], AF.Sigmoid,
                                     bias=lrb_sb[0:1, 0:1], accum_out=lsum[:])
                fparts = spool.tile([1, 2], F32, name=f"fparts{j}", tag="fparts")
                for b in range(B):
                    r = spool.tile([1, 1], F32, name=f"zfr{j}_{b}", tag=f"zfr{b}")
                    nc.vector.tensor_reduce(r[:], g2b[0:1, b * C:(b + 1) * C],
                                            mybir.AxisListType.X, ALU.add)
                    nc.scalar.activation(fparts[:, b:b + 1], r[:], AF.Sigmoid,
                                         bias=fgb_sb[0:1, 0:1], scale=1.0 / C)
                f11 = spool.tile([1, 1], F32, name=f"f11_{j}", tag="f11")
                nc.vector.tensor_reduce(f11[:], fparts[:], mybir.AxisListType.X,
                                        ALU.add)
                nc.vector.tensor_scalar_mul(f11[:], f11[:], 0.5)
                cn11 = spool.tile([1, 1], F32, name=f"cn{j}", tag="cn11")
                nc.vector.tensor_tensor(cn11[:], c11[:], f11[:], ALU.mult)
                rcn = spool.tile([1, 1], F32, name=f"rcn{j}", tag="rcn")
                nc.vector.reciprocal(rcn[:], cn11[:])
                negs = spool.tile([1, 1], F32, name=f"negs{j}", tag="negs")
                nc.vector.tensor_tensor(negs[:], lsum[:], rcn[:], ALU.mult)
                nc.vector.tensor_scalar_mul(negs[:], negs[:],
                                            float(-LR_MEMORY * 2.0 / (T * D) / T))
                negs8 = spool.tile([1, 1], F32, name=f"negs8_{j}", tag="negs8")
                nc.vector.tensor_scalar_mul(negs8[:], negs[:], 1.0 / 8.0)
                negs0 = spool.tile([1, 1], F32, name=f"negs0_{j}", tag="negs0")
                nc.vector.tensor_tensor(negs0[:], negs[:], c11[:], ALU.mult)
                negs_bc = spool.tile([128, 1], F32, name=f"negsbc{j}", tag="negs_bc")
                nc.gpsimd.partition_broadcast(negs_bc[:], negs[:])
                negs0_bc = spool.tile([128, 1], F32, name=f"negs0bc{j}",
                                      tag="negs0_bc")
                nc.gpsimd.partition_broadcast(negs0_bc[:], negs0[:])
                cn_bc = spool.tile([128, 1], F32, name=f"cnbc{j}", tag="cn_bc")
                nc.gpsimd.partition_broadcast(cn_bc[:], cn11[:])
                return cn11, cn_bc, negs_bc, negs0_bc, negs, negs8, negs0

            def upd_q0t_gb0(xb_c, dhp_c, n0bc, n0, j):
                """gW0 = x^T dh' ; q0t += negs0*gW0 ; gb0 -> bk0 row0.
                [128,1024] waves (4 d-tiles each) -> 4 DVE evacs."""
                for w in range(4):
                    ps = psB.tile([128, 4 * MS], F32, name=f"g0{j}_{w}", tag="psB")
                    for u in range(4):
                        i = 4 * w + u
                        sl = slice(u * MS, (u + 1) * MS)
                        for t in range(KT):
                            nc.tensor.matmul(
                                ps[:, sl],
                                xb_c[:, t * D + i * 128:t * D + (i + 1) * 128],
                                dhp_c[:, t * MS:(t + 1) * MS],
                                start=(t == 0), stop=(t == KT - 1))
                    sl2 = slice(4 * w * MS, (4 * w + 4) * MS)
                    nc.vector.scalar_tensor_tensor(q0t[:, sl2], ps[:],
                                                   n0bc[:, 0:1], q0t[:, sl2],
                                                   ALU.mult, ALU.add)
                gb0p = psA.tile([1, MS], F32, name=f"gb0_{j}", tag="psA")
                for t in range(KT):
                    nc.tensor.matmul(gb0p[:], ones_col[:],
                                     dhp_c[:, t * MS:(t + 1) * MS],
                                     start=(t == 0), stop=(t == KT - 1))
                nc.vector.scalar_tensor_tensor(bk0[0:1, :], gb0p[:], n0[0:1, 0:1],
                                               bk0[0:1, :], ALU.mult, ALU.add)

            def upd_q1n(dp_p, h_p, nbc_p, j):
                """gW1n = dpred^T h ; q1n += negs*gW1n. [128,1024] waves."""
                for w in range(4):
                    ps = psB.tile([128, 4 * MS], F32, name=f"g1n{j}_{w}", tag="psB")
                    for u in range(4):
                        i = 4 * w + u
                        sl = slice(u * MS, (u + 1) * MS)
                        for t in range(KT):
                            nc.tensor.matmul(
                                ps[:, sl],
                                dp_p[:, t * D + i * 128:t * D + (i + 1) * 128],
                                h_p[:, t * MS:(t + 1) * MS],
                                start=(t == 0), stop=(t == KT - 1))
                    sl2 = slice(4 * w * MS, (4 * w + 4) * MS)
                    nc.vector.scalar_tensor_tensor(q1n[:, sl2], ps[:],
                                                   nbc_p[:, 0:1], q1n[:, sl2],
                                                   ALU.mult, ALU.add)

            def upd_gb1(dp_c, n8_c, j):
                """gb1 = 1^T dpred ; bk1 row0 += negs8*gb1 (before next mm2)."""
                for n in range(4):
                    gb1p = psA.tile([1, 512], F32, name=f"gb1_{j}_{n}", tag="psA")
                    for t in range(KT):
                        nc.tensor.matmul(
                            gb1p[:], ones_col[:],
                            dp_c[:, t * D + n * 512:t * D + (n + 1) * 512],
                            start=(t == 0), stop=(t == KT - 1))
                    nc.vector.scalar_tensor_tensor(
                        bk1[0:1, n * 512:(n + 1) * 512], gb1p[:], n8_c[0:1, 0:1],
                        bk1[0:1, n * 512:(n + 1) * 512], ALU.mult, ALU.add)

            def upd_q1t(h_c, dp_c, nbc_c, j):
                """gW1t = h^T dpred (m-partitioned) ; q1t += negs*gW1t."""
                for k in range(KM):
                    for w in range(2):
                        ps = psB.tile([128, 1024], F32, name=f"g1t{j}_{k}_{w}",
                                      tag="psB")
                        for s in range(2):
                            n = 2 * w + s
                            sl = slice(s * 512, (s + 1) * 512)
                            for t in range(KT):
                                nc.tensor.matmul(
                                    ps[:, sl],
                                    h_c[:, t * MS + k * 128:t * MS + (k + 1) * 128],
                                    dp_c[:, t * D + n * 512:t * D + (n + 1) * 512],
                                    start=(t == 0), stop=(t == KT - 1))
                        sl2 = slice(k * D + w * 1024, k * D + (w + 1) * 1024)
                        nc.vector.scalar_tensor_tensor(q1t[:, sl2], ps[:],
                                                       nbc_c[:, 0:1], q1t[:, sl2],
                                                       ALU.mult, ALU.add)

            def reforward(xT_p, cb, j):
                """out_j = mlp(P_{j+1}, x_j) -> outp chunk j. cb holds c_{j+1}."""
                ps = mm1_psum(xT_p, "o", j, psB, "psB")
                h2 = otp.tile([128, KT * MS], BF, name=f"h2_{j}", tag="h2")
                nc.scalar.activation(h2[:], ps[:], AF.Silu, scale=cb[:, 0:1])
                hT2 = transpose_h(h2, j, "o")
                osb = otp.tile([128, KT * D], BF, name=f"osb{j}", tag="osb")
                for t in range(KT):
                    for w in range(2):
                        pso = psB.tile([128, 1024], F32, name=f"po{j}_{t}_{w}",
                                       tag="psB")
                        for s in range(2):
                            n = 2 * w + s
                            sl = slice(s * 512, (s + 1) * 512)
                            for k in range(KM):
                                nc.tensor.matmul(
                                    pso[:, sl],
                                    hT2[:, k * T + t * 128:k * T + (t + 1) * 128],
                                    q1t[:, k * D + n * 512:k * D + (n + 1) * 512],
                                    start=(k == 0), stop=False)
                            nc.tensor.matmul(pso[:, sl], ones_row[:],
                                             bk1[:, n * 512:(n + 1) * 512],
                                             start=False, stop=True)
                        sl2 = slice(t * D + w * 1024, t * D + (w + 1) * 1024)
                        nc.scalar.activation(osb[:, sl2], pso[:], AF.Copy,
                                             scale=cb[:, 0:1])
                nc.gpsimd.dma_start(op4[j],
                                    osb.rearrange("p (t d) -> p t d", d=D))

            # ================= prologue: chunk 0 =================
            xb_c, xT_c = load_x(0)
            xb_n, xT_n = load_x(1)
            xb8_c = make_x8(xb_c, 0)

            z0ps = mm1_psum(xT_c, "p", 0, psZ, "psZ")
            h_c = hpool.tile([128, KT * MS], BF, name="h0", tag="h")
            nc.scalar.activation(h_c[:], z0ps[:], AF.Silu, scale=c_bc[:, 0:1])
            hT_c = transpose_h(h_c, 0, "p")
            asb = mm2_pred(hT_c, c_bc, xb8_c, 0)
            aoA = issue_ars(asb, 0)

            # loop-carried state
            z0ps_c = z0ps
            h_p = dhp_p = dp_p = None
            xb_p = xT_p = None
            sc_p = None
            cb_cur = c_bc

            for j in range(NCH):
                last = j == NCH - 1
                # ============ OFF-phase(j): overlaps AR_j ============
                hp_c = hpool.tile([128, KT * MS], BF, name=f"hp{j}", tag="hp")
                nc.scalar.activation(hp_c[:], z0ps_c[:], AF.Derivative_silu,
                                     scale=cb_cur[:, 0:1])
                sc = gates(xT_c, j)
                cn11, cn_bc, negs_bc, negs0_bc, negs, negs8, negs0 = sc

                if j >= 1:
                    upd_q1n(dp_p, h_p, sc_p[2], j - 1)
                    reforward(xT_p, cb_cur, j - 1)
                if not last and j + 2 < NCH:
                    nxt = load_x(j + 2)
                if not last:
                    xb8_n = make_x8(xb_n, j + 1)

                # ============ ON-chain(j): AR_j arrives ============
                dp_c, dpT_c = read_ar(aoA, j)
                dhps = psA.tile([128, KT * MS], F32, name=f"dhs{j}", tag="psA")
                for t in range(KT):
                    sl = slice(t * MS, (t + 1) * MS)
                    for i in range(KD):
                        nc.tensor.matmul(
                            dhps[:, sl],
                            dpT_c[:, i * T + t * 128:i * T + (t + 1) * 128],
                            q1n[:, i * MS:(i + 1) * MS],
                            start=(i == 0), stop=(i == KD - 1))
                dhp_c = hpool.tile([128, KT * MS], BF, name=f"dhp{j}", tag="dhp")
                nc.vector.tensor_tensor(dhp_c[:], dhps[:], hp_c[:], ALU.mult)

                # layer-1 update then direct mm1 for chunk j+1 (or out_15)
                upd_q0t_gb0(xb_c, dhp_c, negs0_bc, negs0, j)
                upd_q1t(h_c, dp_c, negs_bc, j)
                upd_gb1(dp_c, negs8, j)
                if last:
                    reforward(xT_c, cn_bc, j)
                else:
                    z0ps_n = mm1_psum(xT_n, "p", j + 1, psZ, "psZ")
                    h_n = hpool.tile([128, KT * MS], BF, name=f"h{j + 1}", tag="h")
                    nc.scalar.activation(h_n[:], z0ps_n[:], AF.Silu,
                                         scale=cn_bc[:, 0:1])
                    hT_n = transpose_h(h_n, j + 1, "p")
                    asb = mm2_pred(hT_n, cn_bc, xb8_n, j + 1)
                    aoA = issue_ars(asb, j + 1)

                    h_p, dhp_p, dp_p = h_c, dhp_c, dp_c
                    xb_p, xT_p = xb_c, xT_c
                    xb_c, xT_c = xb_n, xT_n
                    if j + 2 < NCH:
                        xb_n, xT_n = nxt
                    xb8_c = xb8_n
                    h_c, hT_c = h_n, hT_n
                    z0ps_c = z0ps_n
                    sc_p = sc
                    c11, cb_cur = cn11, cn_bc
    nc.compile()
    return nc


_NC_CACHE = None


def _get_nc():
    global _NC_CACHE
    if _NC_CACHE is None:
        _NC_CACHE = build()
    return _NC_CACHE


def _bf16(a):
    import ml_dtypes
    return np.asarray(a, np.float32).astype(ml_dtypes.bfloat16)


def make_in_maps(x, W0, b0, W1, b1, lr_w, lr_b, fg_w, fg_b):
    x = np.ascontiguousarray(np.asarray(x, np.float32))
    W0 = np.asarray(W0, np.float32)
    W1 = np.asarray(W1, np.float32)
    xb16 = _bf16(x)
    xt16 = np.ascontiguousarray(_bf16(x).transpose(2, 0, 1))   # [D, B, L]
    lrfg = np.ascontiguousarray(
        np.stack([np.asarray(lr_w, np.float32)[0],
                  np.asarray(fg_w, np.float32)[0]], axis=1))
    ident = np.eye(128, dtype=np.float32)
    in_maps = []
    for s in range(NCORES):
        sl = slice(s * MS, (s + 1) * MS)
        in_maps.append({
            "xb16": xb16,
            "xt16": xt16,
            "w0t": _bf16(np.ascontiguousarray(W0[sl, :].T)),
            "w1t": _bf16(np.ascontiguousarray(W1[:, sl].T)),
            "w1n": _bf16(np.ascontiguousarray(W1[:, sl])),
            "lrfg": _bf16(lrfg),
            "lrb": np.asarray(lr_b, np.float32).reshape(1, 1),
            "fgb": np.asarray(fg_b, np.float32).reshape(1, 1),
            "b0": _bf16(np.asarray(b0, np.float32)[sl].reshape(1, MS)),
            "b1d8": _bf16((np.asarray(b1, np.float32) / 8.0).reshape(1, D)),
            "ident": _bf16(ident),
        })
    return in_maps


def run(inputs, **kw):
    nc = _get_nc()
    in_maps = make_in_maps(**inputs)
    res = run_bass_kernel_spmd(nc, in_maps, core_ids=list(range(NCORES)), **kw)
    out = np.zeros((B, L, D), np.float32)
    for r in res.results:
        out += np.asarray(r["outp"], dtype=np.float32)
    return out, res


def kernel(**inputs) -> np.ndarray:
    out, _ = run(inputs)
    return out


# revision 17
# speedup vs baseline: 2.8195x; 1.4277x over previous
"""NeuralMemory fast-weight recurrence on 8 Trainium2 NeuronCores.

Sharding: 8-way tensor-parallel over memory dim M=2048 (m_s=256/core).
One bf16 AllReduce per chunk carries the partial pred with x/8
pre-subtracted, so the AR output IS dpred = pred - x; the d-partitioned
copy (dpredT) is read back via hardware XBAR DMA transpose.
Per chunk the serial chain is:
  AR_j -> dh' -> gW0/q0t update -> mm1_{j+1} -> silu -> hT ->
  gW1t/q1t + gb1 updates -> mm2 pred_{j+1} -> AR_{j+1}
while gW1n/q1n update, out_{j-1} re-forward, gates, and x prefetch run
inside the AllReduce wait window.

- Host passes x pre-converted to bf16 in BOTH layouts (xb16 [B,L,D],
  xt16 [D,B,L]) - no casting DMAs, no PE transposes for x.
- Weights kept in Q-space (divided by running forget product c) so each
  update is one fused scaled-add reading the gradient PSUM.
- Gradient/update PSUMs use [128,1024] waves to halve DVE evac count.
- Output partials stored bf16; host sums 8 cores in f32.
"""
import numpy as np
import concourse.bacc as bacc
import concourse.mybir as mybir
import concourse.tile as tile
from concourse.bass_utils import run_bass_kernel_spmd

BF = mybir.dt.bfloat16
F32 = mybir.dt.float32
AF = mybir.ActivationFunctionType
ALU = mybir.AluOpType

NCORES = 8
B, L, D, M = 2, 2048, 2048, 2048
C = 128                 # reference CHUNK
NCH = L // C            # 16 chunks
T = B * C               # 256 tokens per chunk
MS = M // NCORES        # 256 per-core memory slice
KD = D // 128           # 16 tiles over D
KT = T // 128           # 2 tiles over tokens
KM = MS // 128          # 2 tiles over m_s
LR_MEMORY = 0.01


def build(no_ar=False):
    nc = bacc.Bacc("TRN2", target_bir_lowering=False, num_devices=NCORES)
    xb16_in = nc.dram_tensor("xb16", [B, L, D], BF, kind="ExternalInput")
    xt16_in = nc.dram_tensor("xt16", [D, B, L], BF, kind="ExternalInput")
    w0t_in = nc.dram_tensor("w0t", [D, MS], BF, kind="ExternalInput")
    w1t_in = nc.dram_tensor("w1t", [MS, D], BF, kind="ExternalInput")
    w1n_in = nc.dram_tensor("w1n", [D, MS], BF, kind="ExternalInput")
    lrfg_in = nc.dram_tensor("lrfg", [D, 2], BF, kind="ExternalInput")
    lrb_in = nc.dram_tensor("lrb", [1, 1], F32, kind="ExternalInput")
    fgb_in = nc.dram_tensor("fgb", [1, 1], F32, kind="ExternalInput")
    b0_in = nc.dram_tensor("b0", [1, MS], BF, kind="ExternalInput")
    b1d8_in = nc.dram_tensor("b1d8", [1, D], BF, kind="ExternalInput")
    ident_in = nc.dram_tensor("ident", [128, 128], BF, kind="ExternalInput")
    outp = nc.dram_tensor("outp", [B, L, D], BF, kind="ExternalOutput")

    xb4 = xb16_in.rearrange("b (nj p) d -> nj p b d", p=C)        # [NCH,128,B,D]
    xt4 = xt16_in.rearrange("(i p) b (nj c) -> nj p i b c", p=128, c=C)
    op4 = outp.rearrange("b (nj p) d -> nj p b d", p=C)

    with tile.TileContext(nc) as tc:
        with (
            tc.tile_pool(name="wp", bufs=1) as wp,            # persistent weights
            tc.tile_pool(name="xp", bufs=4) as xp,            # x chunk streams
            tc.tile_pool(name="x8p", bufs=2) as x8p,          # x/8 staging
            tc.tile_pool(name="hp_", bufs=2) as hpool,        # h/hp/dhp
            tc.tile_pool(name="htp", bufs=2) as htp,          # hT
            tc.tile_pool(name="dpp", bufs=2) as dpp,          # dpred/dpredT sbuf
            tc.tile_pool(name="arp", bufs=1) as arp,          # arin staging sbuf
            tc.tile_pool(name="otp", bufs=1) as otp,          # out/h2/hT2 staging
            tc.tile_pool(name="sp", bufs=2) as spool,         # tiny scalar tiles
            tc.tile_pool(name="psB", bufs=2, space="PSUM") as psB,  # [128,1024]
            tc.tile_pool(name="psA", bufs=2, space="PSUM") as psA,  # [128,512]
            tc.tile_pool(name="psZ", bufs=1, space="PSUM") as psZ,  # z0 accum
            tc.tile_pool(name="psT", bufs=1, space="PSUM") as psT,  # transposes
            tc.tile_pool(name="dr", bufs=2, space="DRAM") as dr,
        ):
            # ---------------- persistent weights (bf16, Q-space) ----------------
            q0t = wp.tile([128, KD * MS], BF, name="q0t")     # W0^T: d-tile i at i*MS
            q1t = wp.tile([128, KM * D], BF, name="q1t")      # W1^T: m-tile k at k*D
            q1n = wp.tile([128, KD * MS], BF, name="q1n")     # W1:   d-tile i at i*MS
            bk0 = wp.tile([128, MS], BF, name="bk0")          # row0 = qb0
            bk1 = wp.tile([128, D], BF, name="bk1")           # row0 = qb1/8
            lrfg = wp.tile([128, KD * 2], BF, name="lrfg")
            ones_row = wp.tile([128, 128], BF, name="ones_row")  # row0 = 1, rest 0
            ones_col = wp.tile([128, 1], BF, name="ones_col")    # all ones
            onesT = wp.tile([1, T], BF, name="onesT")            # [1,256] ones
            ident = wp.tile([128, 128], BF, name="ident")
            lrb_sb = wp.tile([1, 1], F32, name="lrb_sb")
            fgb_sb = wp.tile([1, 1], F32, name="fgb_sb")

            nc.sync.dma_start(q0t.rearrange("p (i m) -> p i m", m=MS),
                              w0t_in.rearrange("(i p) m -> p i m", p=128))
            nc.sync.dma_start(q1n.rearrange("p (i m) -> p i m", m=MS),
                              w1n_in.rearrange("(i p) m -> p i m", p=128))
            nc.sync.dma_start(q1t.rearrange("p (k d) -> p k d", d=D),
                              w1t_in.rearrange("(k p) d -> p k d", p=128))
            nc.sync.dma_start(lrfg.rearrange("p (i g) -> p i g", g=2),
                              lrfg_in.rearrange("(i p) g -> p i g", p=128))
            nc.gpsimd.memset(bk0[:], 0.0)
            nc.gpsimd.memset(bk1[:], 0.0)
            nc.sync.dma_start(bk0[0:1, :], b0_in[:])
            nc.sync.dma_start(bk1[0:1, :], b1d8_in[:])
            nc.gpsimd.memset(ones_row[:], 0.0)
            nc.vector.memset(ones_row[0:1, :], 1.0)
            nc.vector.memset(ones_col[:], 1.0)
            nc.vector.memset(onesT[:], 1.0)
            nc.sync.dma_start(ident[:], ident_in[:])
            nc.sync.dma_start(lrb_sb[:], lrb_in[:])
            nc.sync.dma_start(fgb_sb[:], fgb_in[:])

            # running forget product c (scalar state), starts at 1
            c11 = spool.tile([1, 1], F32, name="c11")
            c_bc = spool.tile([128, 1], F32, name="c_bc")
            nc.vector.memset(c11[:], 1.0)
            nc.gpsimd.partition_broadcast(c_bc[:], c11[:])

            # ---------------- helpers ----------------
            def load_x(j):
                xb = xp.tile([128, KT * D], BF, name=f"xb{j}", tag="xb")
                xT = xp.tile([128, KD * T], BF, name=f"xT{j}", tag="xT")
                nc.sync.dma_start(xb.rearrange("p (t d) -> p t d", d=D), xb4[j])
                xTv = xT.rearrange("p (i b c) -> p i b c", b=B, c=C)
                for b in range(B):
                    nc.sync.dma_start(xTv[:, :, b, :], xt4[j][:, :, b, :])
                return xb, xT

            def make_x8(xb, j):
                xb8 = x8p.tile([128, KT * D], BF, name=f"xb8_{j}", tag="xb8")
                nc.gpsimd.tensor_scalar_mul(xb8[:], xb[:], 0.125)
                return xb8

            def mm1_psum(xT, tag, j, pool, ptag, close=True):
                """z0 Q-space pre-activation [t(2 tiles), MS] in one psum tile.
                close=False leaves the group open for the lagged correction."""
                ps = pool.tile([128, KT * MS], F32, name=f"z0{tag}{j}", tag=ptag)
                for t in range(KT):
                    sl = slice(t * MS, (t + 1) * MS)
                    for i in range(KD):
                        nc.tensor.matmul(ps[:, sl],
                                         xT[:, i * T + t * 128:i * T + (t + 1) * 128],
                                         q0t[:, i * MS:(i + 1) * MS],
                                         start=(i == 0), stop=False)
                    nc.tensor.matmul(ps[:, sl], ones_row[:], bk0[:],
                                     start=False, stop=close)
                return ps

            def transpose_h(h, j, tag):
                hT = htp.tile([128, KM * T], BF, name=f"hT{tag}{j}", tag=f"hT{tag}")
                for t in range(KT):
                    for k in range(KM):
                        pt = psT.tile([128, 128], BF, name=f"ptr{tag}{j}{t}{k}",
                                      tag="psT")
                        nc.tensor.transpose(
                            pt[:], h[:, t * MS + k * 128:t * MS + (k + 1) * 128],
                            ident[:])
                        if (t + k) % 2 == 0:
                            nc.vector.tensor_copy(
                                hT[:, k * T + t * 128:k * T + (t + 1) * 128], pt[:])
                        else:
                            nc.scalar.copy(hT[:, k * T + t * 128:k * T + (t + 1) * 128],
                                           pt[:])
                return hT

            def mm2_pred(hT, cnb, xb8_n, j):
                """pred partial [T, D] with Q-scale cn and -x/8 folded into the
                evac; [128,1024] waves -> 4 DVE evacs."""
                asb = arp.tile([128, KT * D], BF, name=f"arA{j}", tag="arA")
                for t in range(KT):
                    for w in range(2):
                        ps = psB.tile([128, 1024], F32, name=f"pp{j}_{t}_{w}",
                                      tag="psB")
                        for s in range(2):
                            n = 2 * w + s
                            sl = slice(s * 512, (s + 1) * 512)
                            for k in range(KM):
                                nc.tensor.matmul(
                                    ps[:, sl],
                                    hT[:, k * T + t * 128:k * T + (t + 1) * 128],
                                    q1t[:, k * D + n * 512:k * D + (n + 1) * 512],
                                    start=(k == 0), stop=False)
                            nc.tensor.matmul(ps[:, sl], ones_row[:],
                                             bk1[:, n * 512:(n + 1) * 512],
                                             start=False, stop=True)
                        sl2 = slice(t * D + w * 1024, t * D + (w + 1) * 1024)
                        nc.vector.scalar_tensor_tensor(asb[:, sl2], ps[:],
                                                       cnb[:, 0:1], xb8_n[:, sl2],
                                                       ALU.mult, ALU.subtract)
                return asb

            def issue_ars(asb, j):
                arA = dr.tile([T, D], BF, name=f"drA{j}", tag="drA")
                nc.sync.dma_start(arA.rearrange("(t p) d -> p t d", p=128),
                                  asb.rearrange("p (t d) -> p t d", d=D))
                if no_ar:
                    return arA
                aoA = dr.tile([T, D], BF, name=f"aoA{j}", tag="doA",
                              addr_space="Shared")
                nc.gpsimd.collective_compute(
                    "AllReduce", ALU.add, replica_groups=[list(range(NCORES))],
                    ins=[arA.opt()], outs=[aoA.opt()])
                return aoA

            def read_ar(aoA, j):
                dpT = dpp.tile([128, KD * T], BF, name=f"dpT{j}", tag="dpT")
                dp = dpp.tile([128, KT * D], BF, name=f"dp{j}", tag="dp")
                # dpredT tiles via hardware XBAR transpose straight from the
                # AllReduce output (dh' needs d-partitioned dpred first)
                for i in range(KD):
                    eng = nc.scalar if i % 2 == 0 else nc.sync
                    eng.dma_start(dpT[:, i * T:(i + 1) * T],
                                  aoA[:, i * 128:(i + 1) * 128], transpose=True)
                nc.sync.dma_start(dp.rearrange("p (t d) -> p t d", d=D),
                                  aoA.rearrange("(t p) d -> p t d", p=128))
                return dp, dpT

            def gates(xT, j):
                g2a = psA.tile([1, T], F32, name=f"g2a_{j}", tag="psA")
                g2b = psA.tile([1, T], F32, name=f"g2b_{j}", tag="psA")
                for i in range(KD):
                    nc.tensor.matmul(g2a[:], lrfg[:, 2 * i:2 * i + 1],
                                     xT[:, i * T:(i + 1) * T],
                                     start=(i == 0), stop=(i == KD - 1))
                for i in range(KD):
                    nc.tensor.matmul(g2b[:], lrfg[:, 2 * i + 1:2 * i + 2],
                                     xT[:, i * T:(i + 1) * T],
                                     start=(i == 0), stop=(i == KD - 1))
                sigl = spool.tile([1, T], F32, name=f"sigl{j}", tag="sigl")
                lsum = spool.tile([1, 1], F32, name=f"lsum{j}", tag="lsum")
                nc.scalar.activation(sigl[:], g2a[:], AF.Sigmoid,
                                     bias=lrb_sb[0:1, 0:1], accum_out=lsum[:])
                fparts = spool.tile([1, 2], F32, name=f"fparts{j}", tag="fparts")
                for b in range(B):
                    r = spool.tile([1, 1], F32, name=f"zfr{j}_{b}", tag=f"zfr{b}")
                    nc.vector.tensor_reduce(r[:], g2b[0:1, b * C:(b + 1) * C],
                                            mybir.AxisListType.X, ALU.add)
                    nc.scalar.activation(fparts[:, b:b + 1], r[:], AF.Sigmoid,
                                         bias=fgb_sb[0:1, 0:1], scale=1.0 / C)
                f11 = spool.tile([1, 1], F32, name=f"f11_{j}", tag="f11")
                nc.vector.tensor_reduce(f11[:], fparts[:], mybir.AxisListType.X,
                                        ALU.add)
                nc.vector.tensor_scalar_mul(f11[:], f11[:], 0.5)
                cn11 = spool.tile([1, 1], F32, name=f"cn{j}", tag="cn11")
                nc.vector.tensor_tensor(cn11[:], c11[:], f11[:], ALU.mult)
                rcn = spool.tile([1, 1], F32, name=f"rcn{j}", tag="rcn")
                nc.vector.reciprocal(rcn[:], cn11[:])
                negs = spool.tile([1, 1], F32, name=f"negs{j}", tag="negs")
                nc.vector.tensor_tensor(negs[:], lsum[:], rcn[:], ALU.mult)
                nc.vector.tensor_scalar_mul(negs[:], negs[:],
                                            float(-LR_MEMORY * 2.0 / (T * D) / T))
                negs8 = spool.tile([1, 1], F32, name=f"negs8_{j}", tag="negs8")
                nc.vector.tensor_scalar_mul(negs8[:], negs[:], 1.0 / 8.0)
                negs0 = spool.tile([1, 1], F32, name=f"negs0_{j}", tag="negs0")
                nc.vector.tensor_tensor(negs0[:], negs[:], c11[:], ALU.mult)
                negs_bc = spool.tile([128, 1], F32, name=f"negsbc{j}", tag="negs_bc")
                nc.gpsimd.partition_broadcast(negs_bc[:], negs[:])
                negs0_bc = spool.tile([128, 1], F32, name=f"negs0bc{j}",
                                      tag="negs0_bc")
                nc.gpsimd.partition_broadcast(negs0_bc[:], negs0[:])
                cn_bc = spool.tile([128, 1], F32, name=f"cnbc{j}", tag="cn_bc")
                nc.gpsimd.partition_broadcast(cn_bc[:], cn11[:])
                return cn11, cn_bc, negs_bc, negs0_bc, negs, negs8, negs0

            def upd_q0t_gb0(xb_c, dhp_c, n0bc, n0, j):
                """gW0 = x^T dh' ; q0t += negs0*gW0 ; gb0 -> bk0 row0.
                [128,1024] waves (4 d-tiles each) -> 4 DVE evacs."""
                for w in range(4):
                    ps = psB.tile([128, 4 * MS], F32, name=f"g0{j}_{w}", tag="psB")
                    for u in range(4):
                        i = 4 * w + u
                        sl = slice(u * MS, (u + 1) * MS)
                        for t in range(KT):
                            nc.tensor.matmul(
                                ps[:, sl],
                                xb_c[:, t * D + i * 128:t * D + (i + 1) * 128],
                                dhp_c[:, t * MS:(t + 1) * MS],
                                start=(t == 0), stop=(t == KT - 1))
                    sl2 = slice(4 * w * MS, (4 * w + 4) * MS)
                    nc.vector.scalar_tensor_tensor(q0t[:, sl2], ps[:],
                                                   n0bc[:, 0:1], q0t[:, sl2],
                                                   ALU.mult, ALU.add)
                gb0p = psA.tile([1, MS], F32, name=f"gb0_{j}", tag="psA")
                for t in range(KT):
                    nc.tensor.matmul(gb0p[:], ones_col[:],
                                     dhp_c[:, t * MS:(t + 1) * MS],
                                     start=(t == 0), stop=(t == KT - 1))
                nc.vector.scalar_tensor_tensor(bk0[0:1, :], gb0p[:], n0[0:1, 0:1],
                                               bk0[0:1, :], ALU.mult, ALU.add)

            def upd_q1n(dp_p, h_p, nbc_p, j):
                """gW1n = dpred^T h ; q1n += negs*gW1n. [128,1024] waves."""
                for w in range(4):
                    ps = psB.tile([128, 4 * MS], F32, name=f"g1n{j}_{w}", tag="psB")
                    for u in range(4):
                        i = 4 * w + u
                        sl = slice(u * MS, (u + 1) * MS)
                        for t in range(KT):
                            nc.tensor.matmul(
                                ps[:, sl],
                                dp_p[:, t * D + i * 128:t * D + (i + 1) * 128],
                                h_p[:, t * MS:(t + 1) * MS],
                                start=(t == 0), stop=(t == KT - 1))
                    sl2 = slice(4 * w * MS, (4 * w + 4) * MS)
                    nc.vector.scalar_tensor_tensor(q1n[:, sl2], ps[:],
                                                   nbc_p[:, 0:1], q1n[:, sl2],
                                                   ALU.mult, ALU.add)

            def upd_gb1(dp_c, n8_c, j):
                """gb1 = 1^T dpred ; bk1 row0 += negs8*gb1 (before next mm2)."""
                for n in range(4):
                    gb1p = psA.tile([1, 512], F32, name=f"gb1_{j}_{n}", tag="psA")
                    for t in range(KT):
                        nc.tensor.matmul(
                            gb1p[:], ones_col[:],
                            dp_c[:, t * D + n * 512:t * D + (n + 1) * 512],
                            start=(t == 0), stop=(t == KT - 1))
                    nc.vector.scalar_tensor_tensor(
                        bk1[0:1, n * 512:(n + 1) * 512], gb1p[:], n8_c[0:1, 0:1],
                        bk1[0:1, n * 512:(n + 1) * 512], ALU.mult, ALU.add)

            def upd_q1t(h_c, dp_c, nbc_c, j):
                """gW1t = h^T dpred (m-partitioned) ; q1t += negs*gW1t."""
                for k in range(KM):
                    for w in range(2):
                        ps = psB.tile([128, 1024], F32, name=f"g1t{j}_{k}_{w}",
                                      tag="psB")
                        for s in range(2):
                            n = 2 * w + s
                            sl = slice(s * 512, (s + 1) * 512)
                            for t in range(KT):
                                nc.tensor.matmul(
                                    ps[:, sl],
                                    h_c[:, t * MS + k * 128:t * MS + (k + 1) * 128],
                                    dp_c[:, t * D + n * 512:t * D + (n + 1) * 512],
                                    start=(t == 0), stop=(t == KT - 1))
                        sl2 = slice(k * D + w * 1024, k * D + (w + 1) * 1024)
                        nc.vector.scalar_tensor_tensor(q1t[:, sl2], ps[:],
                                                       nbc_c[:, 0:1], q1t[:, sl2],
                                                       ALU.mult, ALU.add)

            def reforward(xT_p, cb, j):
                """out_j = mlp(P_{j+1}, x_j) -> outp chunk j. cb holds c_{j+1}."""
                ps = mm1_psum(xT_p, "o", j, psB, "psB")
                h2 = otp.tile([128, KT * MS], BF, name=f"h2_{j}", tag="h2")
                nc.scalar.activation(h2[:], ps[:], AF.Silu, scale=cb[:, 0:1])
                hT2 = transpose_h(h2, j, "o")
                osb = otp.tile([128, KT * D], BF, name=f"osb{j}", tag="osb")
                for t in range(KT):
                    for w in range(2):
                        pso = psB.tile([128, 1024], F32, name=f"po{j}_{t}_{w}",
                                       tag="psB")
                        for s in range(2):
                            n = 2 * w + s
                            sl = slice(s * 512, (s + 1) * 512)
                            for k in range(KM):
                                nc.tensor.matmul(
                                    pso[:, sl],
                                    hT2[:, k * T + t * 128:k * T + (t + 1) * 128],
                                    q1t[:, k * D + n * 512:k * D + (n + 1) * 512],
                                    start=(k == 0), stop=False)
                            nc.tensor.matmul(pso[:, sl], ones_row[:],
                                             bk1[:, n * 512:(n + 1) * 512],
                                             start=False, stop=True)
                        sl2 = slice(t * D + w * 1024, t * D + (w + 1) * 1024)
                        nc.scalar.activation(osb[:, sl2], pso[:], AF.Copy,
                                             scale=cb[:, 0:1])
                nc.gpsimd.dma_start(op4[j],
                                    osb.rearrange("p (t d) -> p t d", d=D))

            # ================= prologue: chunk 0 =================
            xb_c, xT_c = load_x(0)
            xb_n, xT_n = load_x(1)
            xb8_c = make_x8(xb_c, 0)

            z0ps = mm1_psum(xT_c, "p", 0, psZ, "psZ")
            h_c = hpool.tile([128, KT * MS], BF, name="h0", tag="h")
            nc.scalar.activation(h_c[:], z0ps[:], AF.Silu, scale=c_bc[:, 0:1])
            hT_c = transpose_h(h_c, 0, "p")
            asb = mm2_pred(hT_c, c_bc, xb8_c, 0)
            aoA = issue_ars(asb, 0)

            # loop-carried state
            z0ps_c = z0ps
            h_p = dhp_p = dp_p = None
            xb_p = xT_p = None
            sc_p = None
            cb_cur = c_bc

            for j in range(NCH):
                last = j == NCH - 1
                # ============ OFF-phase(j): overlaps AR_j ============
                hp_c = hpool.tile([128, KT * MS], BF, name=f"hp{j}", tag="hp")
                nc.scalar.activation(hp_c[:], z0ps_c[:], AF.Derivative_silu,
                                     scale=cb_cur[:, 0:1])
                sc = gates(xT_c, j)
                cn11, cn_bc, negs_bc, negs0_bc, negs, negs8, negs0 = sc

                if j >= 1:
                    upd_q0t_gb0(xb_p, dhp_p, sc_p[3], sc_p[6], j - 1)
                    upd_q1n(dp_p, h_p, sc_p[2], j - 1)
                    reforward(xT_p, cb_cur, j - 1)
                if not last and j + 2 < NCH:
                    nxt = load_x(j + 2)
                if not last:
                    xb8_n = make_x8(xb_n, j + 1)
                    # XXTs_j = negs0_j * (x_j x_{j+1}^T + 1)   [t, t'] layout
                    xxs = hpool.tile([128, KT * T], BF, name=f"xxs{j}", tag="xxs")
                    psx = psA.tile([128, KT * T], F32, name=f"xx{j}", tag="psA")
                    for t in range(KT):
                        sl = slice(t * T, (t + 1) * T)
                        for i in range(KD):
                            nc.tensor.matmul(
                                psx[:, sl],
                                xT_c[:, i * T + t * 128:i * T + (t + 1) * 128],
                                xT_n[:, i * T:(i + 1) * T],
                                start=(i == 0), stop=False)
                        # +1 everywhere (absorbs the gb0 rank-1 term)
                        nc.tensor.matmul(psx[:, sl], onesT[0:1, 0:128],
                                         onesT[0:1, :], start=False, stop=True)
                    nc.scalar.activation(xxs[:], psx[:], AF.Copy,
                                         scale=negs0_bc[:, 0:1])
                    # z0preQ_{j+1} with OLD weights Q0_j (left open for corr)
                    z0ps_n = mm1_psum(xT_n, "p", j + 1, psZ, "psZ", close=False)

                # ============ ON-chain(j): AR_j arrives ============
                dp_c, dpT_c = read_ar(aoA, j)
                dhps = psA.tile([128, KT * MS], F32, name=f"dhs{j}", tag="psA")
                for t in range(KT):
                    sl = slice(t * MS, (t + 1) * MS)
                    for i in range(KD):
                        nc.tensor.matmul(
                            dhps[:, sl],
                            dpT_c[:, i * T + t * 128:i * T + (t + 1) * 128],
                            q1n[:, i * MS:(i + 1) * MS],
                            start=(i == 0), stop=(i == KD - 1))
                dhp_c = hpool.tile([128, KT * MS], BF, name=f"dhp{j}", tag="dhp")
                nc.vector.tensor_tensor(dhp_c[:], dhps[:], hp_c[:], ALU.mult)

                if last:
                    upd_q0t_gb0(xb_c, dhp_c, negs0_bc, negs0, j)
                    upd_q1t(h_c, dp_c, negs_bc, j)
                    upd_gb1(dp_c, negs8, j)
                    reforward(xT_c, cn_bc, j)
                else:
                    # z0_{j+1} = z0preQ + XXTs_j @ dhp_j (half-lagged W0 update)
                    for tp in range(KT):
                        sl = slice(tp * MS, (tp + 1) * MS)
                        for kt in range(KT):
                            nc.tensor.matmul(
                                z0ps_n[:, sl],
                                xxs[:, kt * T + tp * 128:kt * T + (tp + 1) * 128],
                                dhp_c[:, kt * MS:(kt + 1) * MS],
                                start=False, stop=(kt == KT - 1))
                    h_n = hpool.tile([128, KT * MS], BF, name=f"h{j + 1}", tag="h")
                    nc.scalar.activation(h_n[:], z0ps_n[:], AF.Silu,
                                         scale=cn_bc[:, 0:1])
                    hT_n = transpose_h(h_n, j + 1, "p")
                    upd_q1t(h_c, dp_c, negs_bc, j)
                    upd_gb1(dp_c, negs8, j)
                    asb = mm2_pred(hT_n, cn_bc, xb8_n, j + 1)
                    aoA = issue_ars(asb, j + 1)

                    h_p, dhp_p, dp_p = h_c, dhp_c, dp_c
                    xb_p, xT_p = xb_c, xT_c
                    xb_c, xT_c = xb_n, xT_n
                    if j + 2 < NCH:
                        xb_n, xT_n = nxt
                    xb8_c = xb8_n
                    h_c, hT_c = h_n, hT_n
                    z0ps_c = z0ps_n
                    sc_p = sc
                    c11, cb_cur = cn11, cn_bc
    nc.compile()
    return nc


_NC_CACHE = None


def _get_nc():
    global _NC_CACHE
    if _NC_CACHE is None:
        _NC_CACHE = build()
    return _NC_CACHE


def _bf16(a):
    import ml_dtypes
    return np.asarray(a, np.float32).astype(ml_dtypes.bfloat16)


def make_in_maps(x, W0, b0, W1, b1, lr_w, lr_b, fg_w, fg_b):
    x = np.ascontiguousarray(np.asarray(x, np.float32))
    W0 = np.asarray(W0, np.float32)
    W1 = np.asarray(W1, np.float32)
    xb16 = _bf16(x)
    xt16 = np.ascontiguousarray(_bf16(x).transpose(2, 0, 1))   # [D, B, L]
    lrfg = np.ascontiguousarray(
        np.stack([np.asarray(lr_w, np.float32)[0],
                  np.asarray(fg_w, np.float32)[0]], axis=1))
    ident = np.eye(128, dtype=np.float32)
    in_maps = []
    for s in range(NCORES):
        sl = slice(s * MS, (s + 1) * MS)
        in_maps.append({
            "xb16": xb16,
            "xt16": xt16,
            "w0t": _bf16(np.ascontiguousarray(W0[sl, :].T)),
            "w1t": _bf16(np.ascontiguousarray(W1[:, sl].T)),
            "w1n": _bf16(np.ascontiguousarray(W1[:, sl])),
            "lrfg": _bf16(lrfg),
            "lrb": np.asarray(lr_b, np.float32).reshape(1, 1),
            "fgb": np.asarray(fg_b, np.float32).reshape(1, 1),
            "b0": _bf16(np.asarray(b0, np.float32)[sl].reshape(1, MS)),
            "b1d8": _bf16((np.asarray(b1, np.float32) / 8.0).reshape(1, D)),
            "ident": _bf16(ident),
        })
    return in_maps


def run(inputs, **kw):
    nc = _get_nc()
    in_maps = make_in_maps(**inputs)
    res = run_bass_kernel_spmd(nc, in_maps, core_ids=list(range(NCORES)), **kw)
    out = np.zeros((B, L, D), np.float32)
    for r in res.results:
        out += np.asarray(r["outp"], dtype=np.float32)
    return out, res


def kernel(**inputs) -> np.ndarray:
    out, _ = run(inputs)
    return out
